# revision 46
# baseline (speedup 1.0000x reference)
"""Trainium2 Bass kernel for nn_DualPGD_3092376453437.

Math: the reference's 30-iteration PGD loop collapses in exact arithmetic.
The normalized Sylvester Hadamard Hmat is symmetric and involutive
(H = H^T, H @ H = I), so measure_H == adjoint_H == F with F(Z) = H Z H and
F(F(Z)) = Z.  With GAMMA = 1 the data-fidelity gradient step

    xk <- xk - F(F(xk) - m) = x0          (exact, every iteration)

resets xk to the pseudo-inverse init x0 = F(m), so the TV dual state u sees
the same gradient every iteration and the loop has a closed form.  Further,

    m  = 2*F(w) - F(ones),  w = (x+1)/2
    x0 = F(m) = 2*F(F(w)) - F(F(ones)) = 2*w - ones     (exact)
    z  = (x0 + 1)/2 = w

so z equals w EXACTLY in real arithmetic -- every Hadamard multiply cancels.
The reference's z differs from w only by its own fp32-matmul rounding noise;
computing z := w on device is therefore CLOSER to the fp32 reference than
re-doing the transforms in fp32 (measured: 7.9e-5 absmax on output scale
9.3, i.e. ~8.5e-6 relative -- the fp32 noise floor).  Final closed form
(TAU = 0.25, 30*TAU = 7.5; folded by 2x so w is never materialized):

    vx  = clip(7.5 * D @ x, -2, 2)          (= 2*u_x;  D = row fwd-diff)
    vy  = clip(7.5 * gy(x), -2, 2)          (= 2*u_y;  free-dim fwd-diff)
    out = x - D^T @ vx - (vy - shift_right(vy))

On-device mapping (software-pipelined with LAG=1: per image p the emission
is vx-matmuls(p), elementwise(p), ax-matmuls(p-1), combine(p-1) -- the PE
stream stays one contiguous 48-matmul burst (warm HAM clock, no in-order
PE-queue stalls on the clip) while clips/combines pipeline one image behind):
  - row-direction stencils are PE matmuls with the constant bidiagonal D:
    vx via lhsT = 7.5*D^T (out = lhsT^T @ x = 7.5*D @ x), the adjoint via
    lhsT = D.  The all-zero 128x128 block of D is skipped (3 matmuls per
    pass, each [K=128, M=128, N=256], fp32).
  - column-direction stencils are free-dim-offset vector ops (x75 = 7.5*x
    on ScalarE, shifted diff + clip + combine on VectorE/GpSimd).
  - cost-model timeline: ~30 us per core.  The 48-matmul PE stream is 100%
    dense; HAM warm-up matmuls run during the input-DMA wait; ayn = vy -
    shift(vy) is precomputed off the combine's critical path; DMAs are
    ordered by dependency priority (HWDGE serializes globally).

Sharding: pure data parallel, 8 images per core on 8 NeuronCores.
"""

import numpy as np

import concourse.mybir as mybir
from concourse import bacc
from concourse.bass_utils import run_bass_kernel_spmd
from concourse.tile import TileContext

N_CORES = 8
IMGS = 8  # images per core
P = 128
W = 256
F32 = mybir.dt.float32

_CACHE: dict = {}


def _build():
    nc = bacc.Bacc("TRN2", target_bir_lowering=False, debug=False)

    x_d = nc.dram_tensor("x", [IMGS, W, W], F32, kind="ExternalInput").ap()
    # Hmat is unused by the collapsed algorithm but kept as an input so the
    # binding matches setup_inputs().
    H_d = nc.dram_tensor("Hmat", [W, W], F32, kind="ExternalInput").ap()
    DT75_d = nc.dram_tensor("DT75", [W, W], F32, kind="ExternalInput").ap()
    D_d = nc.dram_tensor("Dmat", [W, W], F32, kind="ExternalInput").ap()
    out_d = nc.dram_tensor("out", [IMGS, W, W], F32, kind="ExternalOutput").ap()

    # row r = po*128 + pi  ->  SBUF layout [pi, po, (img,) w]
    rc = lambda ap: ap.rearrange("(po pi) w -> pi po w", pi=P)
    Copy = mybir.ActivationFunctionType.Copy
    Alu = mybir.AluOpType

    with TileContext(nc) as tc:
        with (
            tc.tile_pool(name="const", bufs=1) as cpool,
            tc.tile_pool(name="sbuf", bufs=1) as pool,
            tc.tile_pool(name="psum", bufs=7, space="PSUM") as ppool,
            tc.tile_pool(name="warmps", bufs=1, space="PSUM") as wpool,
        ):
            DT75_sb = cpool.tile([P, 2, W], F32, tag="DT75")
            D_sb = cpool.tile([P, 2, W], F32, tag="D")
            Hu_sb = cpool.tile([P, 2, W], F32, tag="Hu")  # unused load
            # consts on the scalar-engine HWDGE queue so x(0) on the SP
            # queue isn't stuck behind them (first matmul needs x0 + DT75)
            # HWDGE serializes DMAs globally, so only ORDER matters:
            # DT75 + x0 gate the first matmul -> first; D is needed only at
            # the first ax pass -> after a few images; unused Hmat -> last.
            nc.sync.dma_start(DT75_sb, rc(DT75_d))

            def G_stencil(lhs_sb, dst_ps, rhs_sb, skip):
                # dst = lhs^T @ rhs; skip the all-zero (m,k) block
                for m in range(2):
                    ks = [k for k in range(2) if (m, k) != skip]
                    for j, k in enumerate(ks):
                        nc.tensor.matmul(
                            dst_ps[:, m, :],
                            lhs_sb[:, k, m * P:(m + 1) * P],
                            rhs_sb[:, k, :],
                            start=(j == 0),
                            stop=(j == len(ks) - 1),
                        )

            x_sbs, x75s, vxps, vxs, vys, axps = [], [], [], [], [], []

            # HAM warm-up: dummy matmuls on a zeroed scratch tile run while
            # the input DMAs are still in flight, so the PE clock is already
            # ramped (4/8 -> 8/8) when the real burst starts.  Outputs go to
            # a scratch PSUM bank and are never read.
            zs = pool.tile([P, W], F32, tag="warm")
            nc.gpsimd.memset(zs, 0.0)
            wps = wpool.tile([P, W], F32, tag="warmp")
            for _ in range(3):
                nc.tensor.matmul(wps, zs[:, 0:P], zs, start=True, stop=True)

            # phase 1: all input DMAs in dependency-priority order
            for p in range(IMGS):
                x_sb = pool.tile([P, 2, W], F32, tag=f"x{p}")
                nc.sync.dma_start(x_sb, rc(x_d[p]))
                x_sbs.append(x_sb)
                if p == 2:
                    nc.sync.dma_start(D_sb, rc(D_d))
            nc.sync.dma_start(Hu_sb, rc(H_d))  # unused input, load last

            # phases 2-4 interleaved with lag: PE gets one contiguous
            # 48-matmul stream (vx p, then ax p-LAG), while clips/vy run
            # behind the burst on DVE/Pool.
            LAG = 1
            def emit_elementwise(p):
                x75 = pool.tile([P, 2, W], F32, tag=f"x75_{p}")
                nc.scalar.activation(x75, x_sbs[p], Copy, bias=0.0, scale=7.5)
                x75s.append(x75)
                vx = pool.tile([P, 2, W], F32, tag=f"vx{p}")
                nc.vector.tensor_scalar(vx, vxps[p], -2.0, 2.0,
                                        op0=Alu.max, op1=Alu.min)
                vxs.append(vx)
                # vy in a width-(W+1) pad tile: col 0 = 0, cols 1..W hold
                # vy[0..W-1] (vy[W-1] = 0).  ayn[j] = vy[j] - vy[j-1] is then
                # ONE shifted-slice op, precomputed OFF the combine's
                # critical path (runs during the matmul burst).
                vp = pool.tile([P, 2, W + 1], F32, tag=f"vp{p}")
                nc.gpsimd.memset(vp[:, :, 0:1], 0.0)
                nc.gpsimd.memset(vp[:, :, W:W + 1], 0.0)
                nc.vector.tensor_sub(
                    vp[:, :, 1:W], x75[:, :, 1:W], x75[:, :, 0:W - 1]
                )
                nc.gpsimd.tensor_scalar(vp[:, :, 1:W], vp[:, :, 1:W], -2.0, 2.0,
                                        op0=Alu.max, op1=Alu.min)
                ayn = pool.tile([P, 2, W], F32, tag=f"ay{p}")
                nc.gpsimd.tensor_sub(ayn, vp[:, :, 1:W + 1], vp[:, :, 0:W])
                vys.append(ayn)

            def emit_ax(p):
                axp = ppool.tile([P, 2, W], F32, tag="u")
                G_stencil(D_sb, axp, vxs[p], skip=(0, 1))
                axps.append(axp)

            def emit_combine(p):
                A = pool.tile([P, 2, W], F32, tag=f"A{p}")
                od = rc(out_d[p])
                if p == IMGS - 1:
                    # last image: compute + store per po-half so the first
                    # half's DMA overlaps the second half's compute
                    for h in range(2):
                        nc.vector.scalar_tensor_tensor(
                            A[:, h, :], axps[p][:, h, :], -1.0,
                            x_sbs[p][:, h, :], op0=Alu.mult, op1=Alu.add)
                        nc.vector.tensor_add(A[:, h, :], A[:, h, :],
                                             vys[p][:, h, :])
                        nc.sync.dma_start(od[:, h, :], A[:, h, :])
                else:
                    nc.vector.scalar_tensor_tensor(A, axps[p], -1.0, x_sbs[p],
                                                   op0=Alu.mult, op1=Alu.add)
                    nc.vector.tensor_add(A, A, vys[p])
                    nc.sync.dma_start(od, A)

            for p in range(IMGS):
                vxp = ppool.tile([P, 2, W], F32, tag="u")
                G_stencil(DT75_sb, vxp, x_sbs[p], skip=(1, 0))
                vxps.append(vxp)
                emit_elementwise(p)
                if p >= LAG:
                    emit_ax(p - LAG)
                    emit_combine(p - LAG)
            for p in range(IMGS - LAG, IMGS):
                emit_ax(p)
                emit_combine(p)

    nc.compile()
    return nc


def _consts():
    D = np.zeros((W, W), np.float32)
    for i in range(W - 1):
        D[i, i] = -1.0
        D[i, i + 1] = 1.0
    DT75 = np.ascontiguousarray((7.5 * D.T).astype(np.float32))
    return D, DT75


def _in_maps(x, Hmat):
    xf = np.ascontiguousarray(np.asarray(x, np.float32).reshape(-1, W, W))
    Hm = np.ascontiguousarray(np.asarray(Hmat, np.float32))
    D, DT75 = _consts()
    per = xf.shape[0] // N_CORES
    return [
        {"x": xf[i * per:(i + 1) * per], "Hmat": Hm, "DT75": DT75, "Dmat": D}
        for i in range(N_CORES)
    ]


def kernel(x: np.ndarray, Hmat: np.ndarray) -> np.ndarray:
    if "nc" not in _CACHE:
        _CACHE["nc"] = _build()
    res = run_bass_kernel_spmd(_CACHE["nc"], _in_maps(x, Hmat), list(range(N_CORES)))
    out = np.concatenate([res.results[i]["out"] for i in range(N_CORES)], axis=0)
    return np.ascontiguousarray(out.reshape(x.shape).astype(np.float32))


def profile(np_inputs, tmpdir=None):
    """Run once with NTFF tracing; returns exec_time_ns (or None)."""
    if "nc" not in _CACHE:
        _CACHE["nc"] = _build()
    res = run_bass_kernel_spmd(
        _CACHE["nc"], _in_maps(np_inputs["x"], np_inputs["Hmat"]),
        list(range(N_CORES)), trace=True, tmpdir=tmpdir,
    )
    return res.exec_time_ns


# revision 49
# speedup vs baseline: 1.0108x; 1.0108x over previous
"""Trainium2 Bass kernel for nn_DualPGD_3092376453437.

Math: the reference's 30-iteration PGD loop collapses in exact arithmetic.
The normalized Sylvester Hadamard Hmat is symmetric and involutive
(H = H^T, H @ H = I), so measure_H == adjoint_H == F with F(Z) = H Z H and
F(F(Z)) = Z.  With GAMMA = 1 the data-fidelity gradient step

    xk <- xk - F(F(xk) - m) = x0          (exact, every iteration)

resets xk to the pseudo-inverse init x0 = F(m), so the TV dual state u sees
the same gradient every iteration and the loop has a closed form.  Further,

    m  = 2*F(w) - F(ones),  w = (x+1)/2
    x0 = F(m) = 2*F(F(w)) - F(F(ones)) = 2*w - ones     (exact)
    z  = (x0 + 1)/2 = w

so z equals w EXACTLY in real arithmetic -- every Hadamard multiply cancels.
The reference's z differs from w only by its own fp32-matmul rounding noise;
computing z := w on device is therefore CLOSER to the fp32 reference than
re-doing the transforms in fp32 (measured: 7.9e-5 absmax on output scale
9.3, i.e. ~8.5e-6 relative -- the fp32 noise floor).  Final closed form
(TAU = 0.25, 30*TAU = 7.5; folded by 2x so w is never materialized):

    vx  = clip(7.5 * D @ x, -2, 2)          (= 2*u_x;  D = row fwd-diff)
    vy  = clip(7.5 * gy(x), -2, 2)          (= 2*u_y;  free-dim fwd-diff)
    out = x - D^T @ vx - (vy - shift_right(vy))

On-device mapping (software-pipelined with LAG=1: per image p the emission
is vx-matmuls(p), elementwise(p), ax-matmuls(p-1), combine(p-1) -- the PE
stream stays one contiguous 48-matmul burst (warm HAM clock, no in-order
PE-queue stalls on the clip) while clips/combines pipeline one image behind):
  - row-direction stencils are PE matmuls with the constant bidiagonal D:
    vx via lhsT = 7.5*D^T (out = lhsT^T @ x = 7.5*D @ x), the adjoint via
    lhsT = D.  The all-zero 128x128 block of D is skipped (3 matmuls per
    pass, each [K=128, M=128, N=256], fp32).
  - column-direction stencils are free-dim-offset vector ops (x75 = 7.5*x
    on ScalarE, shifted diff + clip + combine on VectorE/GpSimd).
  - cost-model timeline: ~30 us per core.  The 48-matmul PE stream is 100%
    dense; HAM warm-up matmuls run during the input-DMA wait; ayn = vy -
    shift(vy) is precomputed off the combine's critical path; DMAs are
    ordered by dependency priority (HWDGE serializes globally).

Sharding: pure data parallel, 8 images per core on 8 NeuronCores.
"""

import numpy as np

import concourse.mybir as mybir
from concourse import bacc
from concourse.bass_utils import run_bass_kernel_spmd
from concourse.tile import TileContext

N_CORES = 8
IMGS = 8  # images per core
P = 128
W = 256
F32 = mybir.dt.float32

_CACHE: dict = {}


def _build():
    nc = bacc.Bacc("TRN2", target_bir_lowering=False, debug=False)

    x_d = nc.dram_tensor("x", [IMGS, W, W], F32, kind="ExternalInput").ap()
    # Hmat is unused by the collapsed algorithm but kept as an input so the
    # binding matches setup_inputs().
    H_d = nc.dram_tensor("Hmat", [W, W], F32, kind="ExternalInput").ap()
    DT75_d = nc.dram_tensor("DT75", [W, W], F32, kind="ExternalInput").ap()
    D_d = nc.dram_tensor("Dmat", [W, W], F32, kind="ExternalInput").ap()
    out_d = nc.dram_tensor("out", [IMGS, W, W], F32, kind="ExternalOutput").ap()

    # row r = po*128 + pi  ->  SBUF layout [pi, po, (img,) w]
    rc = lambda ap: ap.rearrange("(po pi) w -> pi po w", pi=P)
    Copy = mybir.ActivationFunctionType.Copy
    Alu = mybir.AluOpType

    with TileContext(nc) as tc:
        with (
            tc.tile_pool(name="const", bufs=1) as cpool,
            tc.tile_pool(name="sbuf", bufs=1) as pool,
            tc.tile_pool(name="psum", bufs=6, space="PSUM") as ppool,
            tc.tile_pool(name="warmps", bufs=2, space="PSUM") as wpool,
        ):
            DT75_sb = cpool.tile([P, 2, W], F32, tag="DT75")
            D_sb = cpool.tile([P, 2, W], F32, tag="D")
            Hu_sb = cpool.tile([P, 2, W], F32, tag="Hu")  # unused load
            # consts on the scalar-engine HWDGE queue so x(0) on the SP
            # queue isn't stuck behind them (first matmul needs x0 + DT75)
            # HWDGE serializes DMAs globally, so only ORDER matters:
            # DT75 + x0 gate the first matmul -> first; D is needed only at
            # the first ax pass -> after a few images; unused Hmat -> last.
            nc.sync.dma_start(DT75_sb, rc(DT75_d))

            def G_stencil(lhs_sb, dst_ps, rhs_sb, skip):
                # dst = lhs^T @ rhs; skip the all-zero (m,k) block
                for m in range(2):
                    ks = [k for k in range(2) if (m, k) != skip]
                    for j, k in enumerate(ks):
                        nc.tensor.matmul(
                            dst_ps[:, m, :],
                            lhs_sb[:, k, m * P:(m + 1) * P],
                            rhs_sb[:, k, :],
                            start=(j == 0),
                            stop=(j == len(ks) - 1),
                        )

            x_sbs, x75s, vxps, vxs, vys, axps = [], [], [], [], [], []

            # HAM warm-up: dummy matmuls on a zeroed scratch tile run while
            # the input DMAs are still in flight, so the PE clock is already
            # ramped (4/8 -> 8/8) when the real burst starts.  Outputs go to
            # a scratch PSUM bank and are never read.
            zs = pool.tile([P, W], F32, tag="warm")
            nc.gpsimd.memset(zs, 0.0)
            wps = wpool.tile([P, W], F32, tag="warmp")
            for _ in range(3):
                nc.tensor.matmul(wps, zs[:, 0:P], zs, start=True, stop=True)

            # phase 1: all input DMAs in dependency-priority order
            for p in range(IMGS):
                x_sb = pool.tile([P, 2, W], F32, tag=f"x{p}")
                nc.sync.dma_start(x_sb, rc(x_d[p]))
                x_sbs.append(x_sb)
                if p == 2:
                    nc.sync.dma_start(D_sb, rc(D_d))
            nc.sync.dma_start(Hu_sb, rc(H_d))  # unused input, load last

            # phases 2-4 interleaved with lag: PE gets one contiguous
            # 48-matmul stream (vx p, then ax p-LAG), while clips/vy run
            # behind the burst on DVE/Pool.
            LAG = 1
            def emit_elementwise(p):
                x75 = pool.tile([P, 2, W], F32, tag=f"x75_{p}")
                nc.scalar.activation(x75, x_sbs[p], Copy, bias=0.0, scale=7.5)
                x75s.append(x75)
                vx = pool.tile([P, 2, W], F32, tag=f"vx{p}")
                nc.vector.tensor_scalar(vx, vxps[p], -2.0, 2.0,
                                        op0=Alu.max, op1=Alu.min)
                vxs.append(vx)
                # vy in a width-(W+1) pad tile: col 0 = 0, cols 1..W hold
                # vy[0..W-1] (vy[W-1] = 0).  ayn[j] = vy[j] - vy[j-1] is then
                # ONE shifted-slice op, precomputed OFF the combine's
                # critical path (runs during the matmul burst).
                vp = pool.tile([P, 2, W + 1], F32, tag=f"vp{p}")
                nc.gpsimd.memset(vp[:, :, 0:1], 0.0)
                nc.gpsimd.memset(vp[:, :, W:W + 1], 0.0)
                nc.vector.tensor_sub(
                    vp[:, :, 1:W], x75[:, :, 1:W], x75[:, :, 0:W - 1]
                )
                nc.gpsimd.tensor_scalar(vp[:, :, 1:W], vp[:, :, 1:W], -2.0, 2.0,
                                        op0=Alu.max, op1=Alu.min)
                ayn = pool.tile([P, 2, W], F32, tag=f"ay{p}")
                nc.gpsimd.tensor_sub(ayn, vp[:, :, 1:W + 1], vp[:, :, 0:W])
                vys.append(ayn)

            def emit_ax(p):
                if p == IMGS - 1:
                    # last image: each po-half of ax in its OWN PSUM bank
                    # (reusing the dead warm-up pool) so the first half's
                    # combine can overlap the second half's matmuls --
                    # same-bank PE-write vs DVE-read would serialize.
                    halves = []
                    for m in range(2):
                        hp = wpool.tile([P, 1, W], F32, tag="warmp")
                        ks = [k for k in range(2) if (m, k) != (0, 1)]
                        for j, k in enumerate(ks):
                            nc.tensor.matmul(
                                hp[:, 0, :],
                                D_sb[:, k, m * P:(m + 1) * P],
                                vxs[p][:, k, :],
                                start=(j == 0),
                                stop=(j == len(ks) - 1),
                            )
                        halves.append(hp)
                    axps.append(halves)
                else:
                    axp = ppool.tile([P, 2, W], F32, tag="u")
                    G_stencil(D_sb, axp, vxs[p], skip=(0, 1))
                    axps.append(axp)

            def emit_combine(p):
                A = pool.tile([P, 2, W], F32, tag=f"A{p}")
                od = rc(out_d[p])
                if p == IMGS - 1:
                    # last image: compute + store per po-half so the first
                    # half's DMA overlaps the second half's compute
                    for h in range(2):
                        nc.vector.scalar_tensor_tensor(
                            A[:, h, :], axps[p][h][:, 0, :], -1.0,
                            x_sbs[p][:, h, :], op0=Alu.mult, op1=Alu.add)
                        nc.vector.tensor_add(A[:, h, :], A[:, h, :],
                                             vys[p][:, h, :])
                        nc.sync.dma_start(od[:, h, :], A[:, h, :])
                else:
                    nc.vector.scalar_tensor_tensor(A, axps[p], -1.0, x_sbs[p],
                                                   op0=Alu.mult, op1=Alu.add)
                    nc.vector.tensor_add(A, A, vys[p])
                    nc.sync.dma_start(od, A)

            for p in range(IMGS):
                vxp = ppool.tile([P, 2, W], F32, tag="u")
                G_stencil(DT75_sb, vxp, x_sbs[p], skip=(1, 0))
                vxps.append(vxp)
                emit_elementwise(p)
                if p >= LAG:
                    emit_ax(p - LAG)
                    emit_combine(p - LAG)
            for p in range(IMGS - LAG, IMGS):
                emit_ax(p)
                emit_combine(p)

    nc.compile()
    return nc


def _consts():
    D = np.zeros((W, W), np.float32)
    for i in range(W - 1):
        D[i, i] = -1.0
        D[i, i + 1] = 1.0
    DT75 = np.ascontiguousarray((7.5 * D.T).astype(np.float32))
    return D, DT75


def _in_maps(x, Hmat):
    xf = np.ascontiguousarray(np.asarray(x, np.float32).reshape(-1, W, W))
    Hm = np.ascontiguousarray(np.asarray(Hmat, np.float32))
    D, DT75 = _consts()
    per = xf.shape[0] // N_CORES
    return [
        {"x": xf[i * per:(i + 1) * per], "Hmat": Hm, "DT75": DT75, "Dmat": D}
        for i in range(N_CORES)
    ]


def kernel(x: np.ndarray, Hmat: np.ndarray) -> np.ndarray:
    if "nc" not in _CACHE:
        _CACHE["nc"] = _build()
    res = run_bass_kernel_spmd(_CACHE["nc"], _in_maps(x, Hmat), list(range(N_CORES)))
    out = np.concatenate([res.results[i]["out"] for i in range(N_CORES)], axis=0)
    return np.ascontiguousarray(out.reshape(x.shape).astype(np.float32))


def profile(np_inputs, tmpdir=None):
    """Run once with NTFF tracing; returns exec_time_ns (or None)."""
    if "nc" not in _CACHE:
        _CACHE["nc"] = _build()
    res = run_bass_kernel_spmd(
        _CACHE["nc"], _in_maps(np_inputs["x"], np_inputs["Hmat"]),
        list(range(N_CORES)), trace=True, tmpdir=tmpdir,
    )
    return res.exec_time_ns
